# revision 45
# baseline (speedup 1.0000x reference)
"""Trainium2 Bass kernel for nn_AttentionCIDNN (block-diagonal crowd attention).

Problem: x[8192, 8, 2] -> last timestep -> 3-layer MLP -> h[8192, 64];
128 groups of 64 agents; per group A = h_g @ h_g^T, column-shifted softmax
P = exp(A - m[j]) / (sum_j exp(A - m[j]) + eps); scatter P onto the block
diagonal of an 8192 x 8192 zero matrix.

Sharding: 8 cores, each owns 1024 contiguous agents (16 groups). The output
is block-diagonal: only the 16 nonzero 64x64 blocks per core are computed.

Key algebra:
- A is bitwise symmetric on the PE, and the reference's m[j] is the row-max,
  so E = exp(A - m[j]) = G^T with G = exp(A - rowmax[i]) -- a PER-PARTITION
  shift. The device ships G; the host pastes each block transposed and
  applies the row normalization E/(sum+eps) during assembly.
- DUAL-TRACK layout: even blocks ride partitions 0:63, odd blocks 64:127,
  via host-packed block-diagonal weights (W1/W2/W3 duplicated on the
  diagonal). Every matmul then moves half the columns (512 instead of 1024)
  at full PE height, and every activation runs at full 128-lane width.
  Attention becomes 8 pair-matmuls with block-diagonal stationary tiles
  diag(h_2p, h_2p+1) producing [A_2p; A_2p+1] stacked.

Structure per core:
- two input DMAs on the gpsimd SWDGE queue, triggered raw right after the
  engine preamble; a dummy activation preloads the scalar ACT table and
  dummy matmuls warm the PE's activity-based clock during the DMA wait.
- all biases fold into the matmuls via ones-rows (L1 additionally K-stacks
  exact bf16 hi/lo splits of x and W1); b3 applies via ACTIVATE's
  per-partition bias.
- L2/L3/attention matmuls are true fp32: exp() amplifies any error in A
  (|A| up to ~168); bf16 or float32r anywhere in that chain pushes max rel
  err past the 2e-2 gate (measured 2.2e-2 with fp32r L2/L3).
- MLP in two 256-col chunks (chunk = 4 block-pairs); activations overlap
  the next chunk's matmuls; attention pairs follow per chunk and softmax
  pieces (rowmax -> subtract -> exp -> DMA) overlap remaining PE work, with
  a single-pair final piece to keep the post-attention tail short.

Self-contained: hardcodes all shapes; builds the Bass graph once per process.
"""

import os
os.environ.setdefault("JAX_PLATFORMS", "axon")  # device exec path under axon

import numpy as np

import concourse.bass as bass
import concourse.bacc as bacc
import concourse.mybir as mybir
from concourse.tile import TileContext
from concourse.bass_utils import run_bass_kernel_spmd

F32 = mybir.dt.float32
BF16 = mybir.dt.bfloat16

BS = 8192          # total agents
NCORES = 8
AGENTS = BS // NCORES   # 1024 agents per core
BLK = 64                # agents per attention group
EPS = np.float32(1e-7)
NP = 8                  # block pairs per core
PCOLS = NP * BLK        # 512 packed columns (pair p covers blocks 2p, 2p+1)

# xws (bf16): [20, 576] = dual-track K-stack: rows 0:10 track A (even
#   blocks), 10:20 track B (odd blocks); within a track the K rows are the
#   exact-f32 split [x_hi;x_lo;x_hi;x_lo;1;1] paired with
#   [W1_hi;W1_hi;W1_lo;W1_lo;b1_hi;b1_lo]. cols 0:512 packed agents,
#   512:576 the block-diagonal L1 weights w1s2 [20, 64].
XWS_COLS = PCOLS + 64
# wb (f32): [128, 257] = W3diag [128, 0:128] | W2a2 rows 0:65 [128:256]
#   | b3b [128, 256:257]
WB_COLS = 257

_NC_CACHE = None
LAST_RESULT = None  # BassKernelResults of the most recent run (for test harness)


def build_nc():
    """Build the single-core Bass graph (identical on all 8 cores)."""
    nc = bacc.Bacc("TRN2", target_bir_lowering=False)

    xws = nc.declare_dram_parameter("xws", [20, XWS_COLS], BF16,
                                    isOutput=False)
    wb = nc.declare_dram_parameter("wb", [128, WB_COLS], F32, isOutput=False)
    ones = nc.declare_dram_parameter("ones", [1, PCOLS], F32, isOutput=False)
    bands = nc.declare_dram_parameter("bands", [128, PCOLS], F32,
                                      isOutput=True)

    # ---- input DMAs on the gpsimd SWDGE queue, emitted raw so they trigger
    # right after the engine preamble instead of behind the tile-pool entry.
    isem = nc.alloc_semaphore("inp")
    xws_s = nc.alloc_sbuf_tensor("xws_s", [20, XWS_COLS], BF16)
    wb_s = nc.alloc_sbuf_tensor("wb_s", [128, WB_COLS], F32)
    nc.gpsimd.dma_start(out=xws_s[:, :], in_=xws[:, :]).then_inc(isem, 16)
    nc.gpsimd.dma_start(out=wb_s[:, :], in_=wb[:, :]).then_inc(isem, 16)

    w1s_s = xws_s[:, PCOLS:PCOLS + 64]
    w3d_s = wb_s[0:128, 0:128]
    w2a_s = wb_s[0:65, 128:256]
    b3b_s = wb_s[0:128, 256:257]

    # scalar: preload the ACT table (1.3us) while the input DMAs fly; the
    # scratch tile is uninitialized, the result is never read.
    scr = nc.alloc_sbuf_tensor("scr", [1, 8], F32)
    scr2 = nc.alloc_sbuf_tensor("scr2", [1, 8], F32)
    nc.scalar.activation(scr2[:, :], scr[:, :],
                         mybir.ActivationFunctionType.Relu)

    # PE warm-up: the PE runs at 1.2 GHz until its free-running activity
    # window sees ~3.4us of continuous matmul traffic, then doubles to
    # 2.4 GHz. Burn the input-DMA wait on dummy matmuls over a zeroed
    # scratch tile so the real fp32 MLP starts warm. Never read back.
    wsrc = nc.alloc_sbuf_tensor("wsrc", [128, 512], BF16)
    wps = nc.alloc_psum_tensor("wps", [128, 512], F32)
    nc.vector.memset(wsrc[:, :], 0.0)
    for _ in range(7):
        nc.tensor.matmul(wps[:, :], wsrc[:, 0:128], wsrc[:, :])

    # xws resident before the first L1 matmul (raw wait: the tile
    # scheduler's deadlock simulator doesn't model raw DMA increments, so
    # this must precede the TileContext). wb needs no explicit wait: the
    # gpsimd DMA queue is FIFO, so the tracked in-tc ones-row DMA (queued
    # after wb) lands after it, and the L2 matmul -- the first wb consumer
    # -- waits on that ones-row via h1a's dependency tracking.
    nc.tensor.wait_ge(isem, 16)

    with TileContext(nc) as tc:
        with (
            tc.tile_pool(name="sb", bufs=1) as sb,
            tc.tile_pool(name="ps", bufs=1, space="PSUM") as ps,
        ):
            pA0 = ps.tile([128, 4 * BLK], F32, name="pA0")   # pairs 0-3
            pA1 = ps.tile([128, 4 * BLK], F32, name="pA1")   # pairs 4-7
            h3 = sb.tile([128, PCOLS], F32)
            h1a = sb.tile([65, PCOLS], F32)
            h2a = sb.tile([128, PCOLS], F32)
            nc.gpsimd.dma_start(out=h1a[64:65, :], in_=ones[:, :])

            # block-diagonal stationary tiles for the attention pairs:
            # diag_p = [[h_2p, 0], [0, h_2p+1]]. Zero the off-diagonal
            # quadrants now (vector is idle during the input wait).
            diag = []
            for p in range(NP):
                dgt = sb.tile([128, 128], F32, name=f"diag{p}")
                diag.append(dgt)
                nc.vector.memset(dgt[:, :], 0.0)

            # MLP in 2 chunks of 256 packed cols (= 4 pairs each)
            MC = PCOLS // 2
            p1 = ps.tile([64, PCOLS], F32, name="p1")
            p2 = {}
            p3 = {}
            for c in range(2):
                sl = slice(c * MC, (c + 1) * MC)
                nc.tensor.matmul(p1[:, sl], w1s_s, xws_s[:, sl])
            # dep-free filler keeps the PE activity window saturated while
            # L2 waits on relu1 (the clock throttles back after idle)
            nc.tensor.matmul(wps[:, :], wsrc[:, 0:128], wsrc[:, :])
            for c in range(2):
                sl = slice(c * MC, (c + 1) * MC)
                nc.vector.tensor_scalar_max(h1a[0:64, sl], p1[:, sl], 0.0)
                p2[c] = ps.tile([128, MC], F32, name=f"p2_{c}")
                nc.tensor.matmul(p2[c], w2a_s, h1a[:, sl])
            for c in range(2):
                sl = slice(c * MC, (c + 1) * MC)
                nc.scalar.activation(h2a[:, sl], p2[c],
                                     mybir.ActivationFunctionType.Relu)
                p3[c] = ps.tile([128, MC], F32, name=f"p3_{c}")
                nc.tensor.matmul(p3[c], w3d_s, h2a[:, sl])

            nc.tensor.matmul(wps[:, :], wsrc[:, 0:128], wsrc[:, :])
            # h3 = p3 + b3 (per-partition bias); chunk 0 on vector (free
            # right after the relu1s) so the first attention pair's diagonal
            # tiles fill while the PE finishes L3 chunk 1.
            nc.vector.tensor_scalar_add(h3[:, 0:MC], p3[0], b3b_s)

            def copies(p):
                cs = slice(p * BLK, (p + 1) * BLK)
                if p == 0:
                    # both pair-0 quadrants on vector: it owns h3 chunk 0
                    # (act3c0) so attention entry never waits on the scalar
                    # queue's position for act3c1
                    nc.vector.tensor_copy(diag[p][0:64, 0:64], h3[0:64, cs])
                else:
                    nc.scalar.activation(
                        diag[p][0:64, 0:64], h3[0:64, cs],
                        mybir.ActivationFunctionType.Identity)
                nc.vector.tensor_copy(diag[p][64:128, 64:128],
                                      h3[64:128, cs])

            def attn(p):
                pa = pA0 if p < 4 else pA1
                nc.tensor.matmul(pa[:, (p % 4) * BLK:(p % 4 + 1) * BLK],
                                 diag[p], h3[:, p * BLK:(p + 1) * BLK])

            for p in range(4):
                copies(p)
            nc.scalar.activation(h3[:, MC:PCOLS], p3[1],
                                 mybir.ActivationFunctionType.Identity,
                                 bias=b3b_s, scale=1.0)
            for p in range(4):
                attn(p)
            for p in range(4, NP):
                copies(p)
                attn(p)

            # softmax pieces in pairs (3+1 per half): the final piece is a
            # single pair so the post-attention chain is short.
            bounds = [0, 4, 8]
            for q in range(2):
                q0, q1 = bounds[q], bounds[q + 1]
                npair = q1 - q0
                qs = slice(q0 * BLK, q1 * BLK)
                pa = (pA0 if q0 < 4 else pA1)[:, (q0 % 4) * BLK:
                                              ((q1 - 1) % 4 + 1) * BLK]
                r_q = sb.tile([128, npair], F32, name=f"r{q}")
                nc.vector.reduce_max(
                    r_q, pa.rearrange("p (b j) -> p b j", j=BLK),
                    axis=mybir.AxisListType.X)
                # G = exp(A - rowmax): per-partition, per-pair shift via a
                # 0-stride broadcast along j
                rrep = bass.AP(tensor=r_q.tensor, offset=r_q.offset,
                               ap=[list(r_q.ap[0]), list(r_q.ap[1]),
                                   [0, BLK]])
                d_q = sb.tile([128, npair * BLK], F32, name=f"d{q}")
                nc.vector.tensor_sub(
                    d_q.rearrange("p (b j) -> p b j", j=BLK),
                    pa.rearrange("p (b j) -> p b j", j=BLK),
                    rrep)
                band_q = sb.tile([128, npair * BLK], F32, name=f"bq{q}")
                nc.scalar.activation(band_q, d_q,
                                     mybir.ActivationFunctionType.Exp)
                nc.gpsimd.dma_start(out=bands[:, qs], in_=band_q)

    nc.compile()
    return nc


def _get_nc():
    global _NC_CACHE
    if _NC_CACHE is None:
        _NC_CACHE = build_nc()
    return _NC_CACHE


def pack_inputs(xt_core, W1, b1, W2, b2, W3, b3):
    import ml_dtypes
    bf = ml_dtypes.bfloat16
    xT = xt_core.T.astype(np.float32)          # [2, 1024]
    x_hi = xT.astype(bf)
    x_lo = (xT - x_hi.astype(np.float32)).astype(bf)
    W1_hi = W1.astype(bf)
    W1_lo = (W1 - W1_hi.astype(np.float32)).astype(bf)
    b1_hi = b1.astype(bf)
    b1_lo = (b1 - b1_hi.astype(np.float32)).astype(bf)

    def track(cols):
        """K=10 exact split stack for the given agent columns."""
        t = np.zeros((10, len(cols)), dtype=bf)
        t[0:2] = x_hi[:, cols]
        t[2:4] = x_lo[:, cols]
        t[4:6] = x_hi[:, cols]
        t[6:8] = x_lo[:, cols]
        t[8:10] = np.ones((2, len(cols)), dtype=bf)
        return t

    idx = np.arange(AGENTS).reshape(16, BLK)
    even = idx[0::2].ravel()                   # track A agent columns
    odd = idx[1::2].ravel()                    # track B
    xws = np.zeros((20, XWS_COLS), dtype=bf)
    xws[0:10, 0:PCOLS] = track(even)
    xws[10:20, 0:PCOLS] = track(odd)
    w1k = np.zeros((10, 32), dtype=bf)
    w1k[0:2] = W1_hi
    w1k[2:4] = W1_hi
    w1k[4:6] = W1_lo
    w1k[6:8] = W1_lo
    w1k[8] = b1_hi
    w1k[9] = b1_lo
    xws[0:10, PCOLS:PCOLS + 32] = w1k          # track A -> out rows 0:32
    xws[10:20, PCOLS + 32:PCOLS + 64] = w1k    # track B -> out rows 32:64

    wb = np.zeros((128, WB_COLS), dtype=np.float32)
    wb[0:64, 0:64] = W3                        # W3diag
    wb[64:128, 64:128] = W3
    wb[0:32, 128:192] = W2                     # W2a2: track A h1 rows
    wb[32:64, 192:256] = W2                    # track B
    wb[64, 128:192] = b2
    wb[64, 192:256] = b2
    wb[0:64, 256] = b3                         # b3b
    wb[64:128, 256] = b3
    return xws, wb


def kernel(x, W1, b1, W2, b2, W3, b3, sub_batches, **run_kwargs):
    global LAST_RESULT
    x = np.asarray(x)
    xt = np.ascontiguousarray(x[:, -1, :], dtype=np.float32)  # [8192, 2]
    W1 = np.asarray(W1, dtype=np.float32)
    W2 = np.asarray(W2, dtype=np.float32)
    W3 = np.asarray(W3, dtype=np.float32)
    b1 = np.asarray(b1, dtype=np.float32)
    b2 = np.asarray(b2, dtype=np.float32)
    b3 = np.asarray(b3, dtype=np.float32)

    ones = np.ones((1, PCOLS), dtype=np.float32)
    in_maps = []
    for d in range(NCORES):
        xws, wb = pack_inputs(
            xt[d * AGENTS:(d + 1) * AGENTS, :], W1, b1, W2, b2, W3, b3)
        in_maps.append({"xws": xws, "wb": wb, "ones": ones})

    nc = _get_nc()
    res = run_bass_kernel_spmd(nc, in_maps, core_ids=list(range(NCORES)),
                               **run_kwargs)
    LAST_RESULT = res

    # Device ships G = exp(A - rowmax) in dual-track layout: pair p has
    # block 2p on partitions 0:64 and block 2p+1 on 64:128. The reference
    # E = exp(A - m[j]) is G^T per block; paste transposed and normalize.
    full = np.zeros((BS, BS), dtype=np.float32)
    for d in range(NCORES):
        bd = np.asarray(res.results[d]["bands"])        # [128, 512] = G
        for p in range(NP):
            for t in range(2):
                n = d * 16 + 2 * p + t                  # global 64-row block
                G = bd[t * 64:(t + 1) * 64, p * BLK:(p + 1) * BLK]
                E = np.ascontiguousarray(G.T)
                P = E / (E.sum(axis=1, keepdims=True) + EPS)
                full[n * BLK:(n + 1) * BLK, n * BLK:(n + 1) * BLK] = P

    starts = np.asarray(sub_batches)[:, 0]
    canonical = np.array_equal(starts, np.arange(128, dtype=np.int64) * BLK)
    if not canonical:
        # General placement: extract the 64x64 blocks and scatter them at the
        # rows given by sub_batches (faithful to the reference .at[].set).
        scat = np.zeros((BS, BS), dtype=np.float32)
        for n in range(128):
            blk = full[n * BLK:(n + 1) * BLK, n * BLK:(n + 1) * BLK]
            rows = int(starts[n]) + np.arange(BLK)
            scat[np.ix_(rows, rows)] = blk
        full = scat
    return full


# revision 46
# speedup vs baseline: 1.0511x; 1.0511x over previous
"""Trainium2 Bass kernel for nn_AttentionCIDNN (block-diagonal crowd attention).

Problem: x[8192, 8, 2] -> last timestep -> 3-layer MLP -> h[8192, 64];
128 groups of 64 agents; per group A = h_g @ h_g^T, column-shifted softmax
P = exp(A - m[j]) / (sum_j exp(A - m[j]) + eps); scatter P onto the block
diagonal of an 8192 x 8192 zero matrix.

Sharding: 8 cores, each owns 1024 contiguous agents (16 groups). The output
is block-diagonal: only the 16 nonzero 64x64 blocks per core are computed.

Key algebra:
- A is bitwise symmetric on the PE, and the reference's m[j] is the row-max,
  so E = exp(A - m[j]) = G^T with G = exp(A - rowmax[i]) -- a PER-PARTITION
  shift. The device ships G; the host pastes each block transposed and
  applies the row normalization E/(sum+eps) during assembly.
- DUAL-TRACK layout: even blocks ride partitions 0:63, odd blocks 64:127,
  via host-packed block-diagonal weights (W1/W2/W3 duplicated on the
  diagonal). Every matmul then moves half the columns (512 instead of 1024)
  at full PE height, and every activation runs at full 128-lane width.
  Attention becomes 8 pair-matmuls with block-diagonal stationary tiles
  diag(h_2p, h_2p+1) producing [A_2p; A_2p+1] stacked.

Structure per core:
- two input DMAs on the gpsimd SWDGE queue, triggered raw right after the
  engine preamble; a dummy activation preloads the scalar ACT table and
  dummy matmuls warm the PE's activity-based clock during the DMA wait.
- all biases fold into the matmuls via ones-rows (L1 additionally K-stacks
  exact bf16 hi/lo splits of x and W1); b3 applies via ACTIVATE's
  per-partition bias.
- L2/L3/attention matmuls are true fp32: exp() amplifies any error in A
  (|A| up to ~168); bf16 or float32r anywhere in that chain pushes max rel
  err past the 2e-2 gate (measured 2.2e-2 with fp32r L2/L3).
- MLP in two 256-col chunks (chunk = 4 block-pairs); activations overlap
  the next chunk's matmuls; attention pairs follow per chunk and softmax
  pieces (rowmax -> subtract -> exp -> DMA) overlap remaining PE work, with
  a single-pair final piece to keep the post-attention tail short.

Self-contained: hardcodes all shapes; builds the Bass graph once per process.
"""

import os
os.environ.setdefault("JAX_PLATFORMS", "axon")  # device exec path under axon

import numpy as np

import concourse.bass as bass
import concourse.bacc as bacc
import concourse.mybir as mybir
from concourse.tile import TileContext
from concourse.bass_utils import run_bass_kernel_spmd

F32 = mybir.dt.float32
BF16 = mybir.dt.bfloat16

BS = 8192          # total agents
NCORES = 8
AGENTS = BS // NCORES   # 1024 agents per core
BLK = 64                # agents per attention group
EPS = np.float32(1e-7)
NP = 8                  # block pairs per core
PCOLS = NP * BLK        # 512 packed columns (pair p covers blocks 2p, 2p+1)

# xws (bf16): [20, 576] = dual-track K-stack: rows 0:10 track A (even
#   blocks), 10:20 track B (odd blocks); within a track the K rows are the
#   exact-f32 split [x_hi;x_lo;x_hi;x_lo;1;1] paired with
#   [W1_hi;W1_hi;W1_lo;W1_lo;b1_hi;b1_lo]. cols 0:512 packed agents,
#   512:576 the block-diagonal L1 weights w1s2 [20, 64].
XWS_COLS = PCOLS + 64
# wb (f32): [128, 257] = W3diag [128, 0:128] | W2a2 rows 0:65 [128:256]
#   | b3b [128, 256:257]
WB_COLS = 257

_NC_CACHE = None
LAST_RESULT = None  # BassKernelResults of the most recent run (for test harness)


def build_nc():
    """Build the single-core Bass graph (identical on all 8 cores)."""
    nc = bacc.Bacc("TRN2", target_bir_lowering=False)

    xws = nc.declare_dram_parameter("xws", [20, XWS_COLS], BF16,
                                    isOutput=False)
    wb = nc.declare_dram_parameter("wb", [128, WB_COLS], F32, isOutput=False)
    ones = nc.declare_dram_parameter("ones", [1, PCOLS], F32, isOutput=False)
    bands = nc.declare_dram_parameter("bands", [128, PCOLS], F32,
                                      isOutput=True)

    # ---- input DMAs on the gpsimd SWDGE queue, emitted raw so they trigger
    # right after the engine preamble instead of behind the tile-pool entry.
    isem = nc.alloc_semaphore("inp")
    xws_s = nc.alloc_sbuf_tensor("xws_s", [20, XWS_COLS], BF16)
    wb_s = nc.alloc_sbuf_tensor("wb_s", [128, WB_COLS], F32)
    nc.gpsimd.dma_start(out=xws_s[:, :], in_=xws[:, :]).then_inc(isem, 16)
    nc.gpsimd.dma_start(out=wb_s[:, :], in_=wb[:, :]).then_inc(isem, 16)

    w1s_s = xws_s[:, PCOLS:PCOLS + 64]
    w3d_s = wb_s[0:128, 0:128]
    w2a_s = wb_s[0:65, 128:256]
    b3b_s = wb_s[0:128, 256:257]

    # scalar: preload the ACT table (1.3us) while the input DMAs fly; the
    # scratch tile is uninitialized, the result is never read.
    scr = nc.alloc_sbuf_tensor("scr", [1, 8], F32)
    scr2 = nc.alloc_sbuf_tensor("scr2", [1, 8], F32)
    nc.scalar.activation(scr2[:, :], scr[:, :],
                         mybir.ActivationFunctionType.Relu)

    # PE warm-up: the PE runs at 1.2 GHz until its free-running activity
    # window sees ~3.4us of continuous matmul traffic, then doubles to
    # 2.4 GHz. Burn the input-DMA wait on dummy matmuls over a zeroed
    # scratch tile so the real fp32 MLP starts warm. Never read back.
    wsrc = nc.alloc_sbuf_tensor("wsrc", [128, 512], BF16)
    wps = nc.alloc_psum_tensor("wps", [128, 512], F32)
    nc.vector.memset(wsrc[:, :], 0.0)
    for _ in range(8):
        nc.tensor.matmul(wps[:, :], wsrc[:, 0:128], wsrc[:, :])

    # xws resident before the first L1 matmul (raw wait: the tile
    # scheduler's deadlock simulator doesn't model raw DMA increments, so
    # this must precede the TileContext). wb needs no explicit wait: the
    # gpsimd DMA queue is FIFO, so the tracked in-tc ones-row DMA (queued
    # after wb) lands after it, and the L2 matmul -- the first wb consumer
    # -- waits on that ones-row via h1a's dependency tracking.
    nc.tensor.wait_ge(isem, 16)

    with TileContext(nc) as tc:
        with (
            tc.tile_pool(name="sb", bufs=1) as sb,
            tc.tile_pool(name="ps", bufs=1, space="PSUM") as ps,
        ):
            pA0 = ps.tile([128, 4 * BLK], F32, name="pA0")   # pairs 0-3
            pA1 = ps.tile([128, 4 * BLK], F32, name="pA1")   # pairs 4-7
            h3 = sb.tile([128, PCOLS], F32)
            h1a = sb.tile([65, PCOLS], F32)
            h2a = sb.tile([128, PCOLS], F32)
            nc.gpsimd.dma_start(out=h1a[64:65, :], in_=ones[:, :])

            # block-diagonal stationary tiles for the attention pairs:
            # diag_p = [[h_2p, 0], [0, h_2p+1]]. Zero the off-diagonal
            # quadrants now (vector is idle during the input wait).
            diag = []
            for p in range(NP):
                dgt = sb.tile([128, 128], F32, name=f"diag{p}")
                diag.append(dgt)
                nc.vector.memset(dgt[:, :], 0.0)

            # MLP in 2 chunks of 256 packed cols (= 4 pairs each)
            MC = PCOLS // 2
            p1 = ps.tile([64, PCOLS], F32, name="p1")
            p2 = {}
            p3 = {}
            for c in range(2):
                sl = slice(c * MC, (c + 1) * MC)
                nc.tensor.matmul(p1[:, sl], w1s_s, xws_s[:, sl])
            # dep-free filler keeps the PE activity window saturated while
            # L2 waits on relu1 (the clock throttles back after idle)
            nc.tensor.matmul(wps[:, :], wsrc[:, 0:128], wsrc[:, :])
            for c in range(2):
                sl = slice(c * MC, (c + 1) * MC)
                nc.vector.tensor_scalar_max(h1a[0:64, sl], p1[:, sl], 0.0)
                p2[c] = ps.tile([128, MC], F32, name=f"p2_{c}")
                nc.tensor.matmul(p2[c], w2a_s, h1a[:, sl])
            for c in range(2):
                sl = slice(c * MC, (c + 1) * MC)
                nc.scalar.activation(h2a[:, sl], p2[c],
                                     mybir.ActivationFunctionType.Relu)
                p3[c] = ps.tile([128, MC], F32, name=f"p3_{c}")
                nc.tensor.matmul(p3[c], w3d_s, h2a[:, sl])

            nc.tensor.matmul(wps[:, :], wsrc[:, 0:128], wsrc[:, :])
            # h3 = p3 + b3 (per-partition bias); chunk 0 on vector (free
            # right after the relu1s) so the first attention pair's diagonal
            # tiles fill while the PE finishes L3 chunk 1.
            nc.vector.tensor_scalar_add(h3[:, 0:MC], p3[0], b3b_s)

            def copies(p):
                cs = slice(p * BLK, (p + 1) * BLK)
                if p == 0:
                    # both pair-0 quadrants on vector: it owns h3 chunk 0
                    # (act3c0) so attention entry never waits on the scalar
                    # queue's position for act3c1
                    nc.vector.tensor_copy(diag[p][0:64, 0:64], h3[0:64, cs])
                else:
                    nc.scalar.activation(
                        diag[p][0:64, 0:64], h3[0:64, cs],
                        mybir.ActivationFunctionType.Identity)
                nc.vector.tensor_copy(diag[p][64:128, 64:128],
                                      h3[64:128, cs])

            def attn(p):
                pa = pA0 if p < 4 else pA1
                nc.tensor.matmul(pa[:, (p % 4) * BLK:(p % 4 + 1) * BLK],
                                 diag[p], h3[:, p * BLK:(p + 1) * BLK])

            for p in range(4):
                copies(p)
            nc.scalar.activation(h3[:, MC:PCOLS], p3[1],
                                 mybir.ActivationFunctionType.Identity,
                                 bias=b3b_s, scale=1.0)
            for p in range(4):
                attn(p)
            for p in range(4, NP):
                copies(p)
                attn(p)

            # softmax pieces in pairs (3+1 per half): the final piece is a
            # single pair so the post-attention chain is short.
            bounds = [0, 4, 8]
            for q in range(2):
                q0, q1 = bounds[q], bounds[q + 1]
                npair = q1 - q0
                qs = slice(q0 * BLK, q1 * BLK)
                pa = (pA0 if q0 < 4 else pA1)[:, (q0 % 4) * BLK:
                                              ((q1 - 1) % 4 + 1) * BLK]
                r_q = sb.tile([128, npair], F32, name=f"r{q}")
                nc.vector.reduce_max(
                    r_q, pa.rearrange("p (b j) -> p b j", j=BLK),
                    axis=mybir.AxisListType.X)
                # G = exp(A - rowmax): per-partition, per-pair shift via a
                # 0-stride broadcast along j
                rrep = bass.AP(tensor=r_q.tensor, offset=r_q.offset,
                               ap=[list(r_q.ap[0]), list(r_q.ap[1]),
                                   [0, BLK]])
                d_q = sb.tile([128, npair * BLK], F32, name=f"d{q}")
                nc.vector.tensor_sub(
                    d_q.rearrange("p (b j) -> p b j", j=BLK),
                    pa.rearrange("p (b j) -> p b j", j=BLK),
                    rrep)
                band_q = sb.tile([128, npair * BLK], F32, name=f"bq{q}")
                nc.scalar.activation(band_q, d_q,
                                     mybir.ActivationFunctionType.Exp)
                nc.gpsimd.dma_start(out=bands[:, qs], in_=band_q)

    nc.compile()
    return nc


def _get_nc():
    global _NC_CACHE
    if _NC_CACHE is None:
        _NC_CACHE = build_nc()
    return _NC_CACHE


def pack_inputs(xt_core, W1, b1, W2, b2, W3, b3):
    import ml_dtypes
    bf = ml_dtypes.bfloat16
    xT = xt_core.T.astype(np.float32)          # [2, 1024]
    x_hi = xT.astype(bf)
    x_lo = (xT - x_hi.astype(np.float32)).astype(bf)
    W1_hi = W1.astype(bf)
    W1_lo = (W1 - W1_hi.astype(np.float32)).astype(bf)
    b1_hi = b1.astype(bf)
    b1_lo = (b1 - b1_hi.astype(np.float32)).astype(bf)

    def track(cols):
        """K=10 exact split stack for the given agent columns."""
        t = np.zeros((10, len(cols)), dtype=bf)
        t[0:2] = x_hi[:, cols]
        t[2:4] = x_lo[:, cols]
        t[4:6] = x_hi[:, cols]
        t[6:8] = x_lo[:, cols]
        t[8:10] = np.ones((2, len(cols)), dtype=bf)
        return t

    idx = np.arange(AGENTS).reshape(16, BLK)
    even = idx[0::2].ravel()                   # track A agent columns
    odd = idx[1::2].ravel()                    # track B
    xws = np.zeros((20, XWS_COLS), dtype=bf)
    xws[0:10, 0:PCOLS] = track(even)
    xws[10:20, 0:PCOLS] = track(odd)
    w1k = np.zeros((10, 32), dtype=bf)
    w1k[0:2] = W1_hi
    w1k[2:4] = W1_hi
    w1k[4:6] = W1_lo
    w1k[6:8] = W1_lo
    w1k[8] = b1_hi
    w1k[9] = b1_lo
    xws[0:10, PCOLS:PCOLS + 32] = w1k          # track A -> out rows 0:32
    xws[10:20, PCOLS + 32:PCOLS + 64] = w1k    # track B -> out rows 32:64

    wb = np.zeros((128, WB_COLS), dtype=np.float32)
    wb[0:64, 0:64] = W3                        # W3diag
    wb[64:128, 64:128] = W3
    wb[0:32, 128:192] = W2                     # W2a2: track A h1 rows
    wb[32:64, 192:256] = W2                    # track B
    wb[64, 128:192] = b2
    wb[64, 192:256] = b2
    wb[0:64, 256] = b3                         # b3b
    wb[64:128, 256] = b3
    return xws, wb


def kernel(x, W1, b1, W2, b2, W3, b3, sub_batches, **run_kwargs):
    global LAST_RESULT
    x = np.asarray(x)
    xt = np.ascontiguousarray(x[:, -1, :], dtype=np.float32)  # [8192, 2]
    W1 = np.asarray(W1, dtype=np.float32)
    W2 = np.asarray(W2, dtype=np.float32)
    W3 = np.asarray(W3, dtype=np.float32)
    b1 = np.asarray(b1, dtype=np.float32)
    b2 = np.asarray(b2, dtype=np.float32)
    b3 = np.asarray(b3, dtype=np.float32)

    ones = np.ones((1, PCOLS), dtype=np.float32)
    in_maps = []
    for d in range(NCORES):
        xws, wb = pack_inputs(
            xt[d * AGENTS:(d + 1) * AGENTS, :], W1, b1, W2, b2, W3, b3)
        in_maps.append({"xws": xws, "wb": wb, "ones": ones})

    nc = _get_nc()
    res = run_bass_kernel_spmd(nc, in_maps, core_ids=list(range(NCORES)),
                               **run_kwargs)
    LAST_RESULT = res

    # Device ships G = exp(A - rowmax) in dual-track layout: pair p has
    # block 2p on partitions 0:64 and block 2p+1 on 64:128. The reference
    # E = exp(A - m[j]) is G^T per block; paste transposed and normalize.
    full = np.zeros((BS, BS), dtype=np.float32)
    for d in range(NCORES):
        bd = np.asarray(res.results[d]["bands"])        # [128, 512] = G
        for p in range(NP):
            for t in range(2):
                n = d * 16 + 2 * p + t                  # global 64-row block
                G = bd[t * 64:(t + 1) * 64, p * BLK:(p + 1) * BLK]
                E = np.ascontiguousarray(G.T)
                P = E / (E.sum(axis=1, keepdims=True) + EPS)
                full[n * BLK:(n + 1) * BLK, n * BLK:(n + 1) * BLK] = P

    starts = np.asarray(sub_batches)[:, 0]
    canonical = np.array_equal(starts, np.arange(128, dtype=np.int64) * BLK)
    if not canonical:
        # General placement: extract the 64x64 blocks and scatter them at the
        # rows given by sub_batches (faithful to the reference .at[].set).
        scat = np.zeros((BS, BS), dtype=np.float32)
        for n in range(128):
            blk = full[n * BLK:(n + 1) * BLK, n * BLK:(n + 1) * BLK]
            rows = int(starts[n]) + np.arange(BLK)
            scat[np.ix_(rows, rows)] = blk
        full = scat
    return full


# revision 47
# speedup vs baseline: 1.0673x; 1.0154x over previous
"""Trainium2 Bass kernel for nn_AttentionCIDNN (block-diagonal crowd attention).

Problem: x[8192, 8, 2] -> last timestep -> 3-layer MLP -> h[8192, 64];
128 groups of 64 agents; per group A = h_g @ h_g^T, column-shifted softmax
P = exp(A - m[j]) / (sum_j exp(A - m[j]) + eps); scatter P onto the block
diagonal of an 8192 x 8192 zero matrix.

Sharding: 8 cores, each owns 1024 contiguous agents (16 groups). The output
is block-diagonal: only the 16 nonzero 64x64 blocks per core are computed.

Key algebra:
- A is bitwise symmetric on the PE, and the reference's m[j] is the row-max,
  so E = exp(A - m[j]) = G^T with G = exp(A - rowmax[i]) -- a PER-PARTITION
  shift. The device ships G; the host pastes each block transposed and
  applies the row normalization E/(sum+eps) during assembly.
- DUAL-TRACK layout: even blocks ride partitions 0:63, odd blocks 64:127,
  via host-packed block-diagonal weights (W1/W2/W3 duplicated on the
  diagonal). Every matmul then moves half the columns (512 instead of 1024)
  at full PE height, and every activation runs at full 128-lane width.
  Attention becomes 8 pair-matmuls with block-diagonal stationary tiles
  diag(h_2p, h_2p+1) producing [A_2p; A_2p+1] stacked.

Structure per core:
- two input DMAs on the gpsimd SWDGE queue, triggered raw right after the
  engine preamble; a dummy activation preloads the scalar ACT table and
  dummy matmuls warm the PE's activity-based clock during the DMA wait.
- all biases fold into the matmuls via ones-rows (L1 additionally K-stacks
  exact bf16 hi/lo splits of x and W1); b3 applies via ACTIVATE's
  per-partition bias.
- L2/L3/attention matmuls are true fp32: exp() amplifies any error in A
  (|A| up to ~168); bf16 or float32r anywhere in that chain pushes max rel
  err past the 2e-2 gate (measured 2.2e-2 with fp32r L2/L3).
- MLP in two 256-col chunks (chunk = 4 block-pairs); activations overlap
  the next chunk's matmuls; attention pairs follow per chunk and softmax
  pieces (rowmax -> subtract -> exp -> DMA) overlap remaining PE work, with
  a single-pair final piece to keep the post-attention tail short.

Self-contained: hardcodes all shapes; builds the Bass graph once per process.
"""

import os
os.environ.setdefault("JAX_PLATFORMS", "axon")  # device exec path under axon

import numpy as np

import concourse.bass as bass
import concourse.bacc as bacc
import concourse.mybir as mybir
from concourse.tile import TileContext
from concourse.bass_utils import run_bass_kernel_spmd

F32 = mybir.dt.float32
BF16 = mybir.dt.bfloat16

BS = 8192          # total agents
NCORES = 8
AGENTS = BS // NCORES   # 1024 agents per core
BLK = 64                # agents per attention group
EPS = np.float32(1e-7)
NP = 8                  # block pairs per core
PCOLS = NP * BLK        # 512 packed columns (pair p covers blocks 2p, 2p+1)

# xws (bf16): [20, 576] = dual-track K-stack: rows 0:10 track A (even
#   blocks), 10:20 track B (odd blocks); within a track the K rows are the
#   exact-f32 split [x_hi;x_lo;x_hi;x_lo;1;1] paired with
#   [W1_hi;W1_hi;W1_lo;W1_lo;b1_hi;b1_lo]. cols 0:512 packed agents,
#   512:576 the block-diagonal L1 weights w1s2 [20, 64].
XWS_COLS = PCOLS + 64
# wb (f32): [128, 257] = W3diag [128, 0:128] | W2a2 rows 0:65 [128:256]
#   | b3b [128, 256:257]
WB_COLS = 257

_NC_CACHE = None
LAST_RESULT = None  # BassKernelResults of the most recent run (for test harness)


def build_nc():
    """Build the single-core Bass graph (identical on all 8 cores)."""
    nc = bacc.Bacc("TRN2", target_bir_lowering=False)

    xws = nc.declare_dram_parameter("xws", [20, XWS_COLS], BF16,
                                    isOutput=False)
    wb = nc.declare_dram_parameter("wb", [128, WB_COLS], F32, isOutput=False)
    ones = nc.declare_dram_parameter("ones", [1, PCOLS], F32, isOutput=False)
    bands = nc.declare_dram_parameter("bands", [128, PCOLS], F32,
                                      isOutput=True)

    # ---- input DMAs on the gpsimd SWDGE queue, emitted raw so they trigger
    # right after the engine preamble instead of behind the tile-pool entry.
    isem = nc.alloc_semaphore("inp")
    xws_s = nc.alloc_sbuf_tensor("xws_s", [20, XWS_COLS], BF16)
    wb_s = nc.alloc_sbuf_tensor("wb_s", [128, WB_COLS], F32)
    nc.gpsimd.dma_start(out=xws_s[:, :], in_=xws[:, :]).then_inc(isem, 16)
    nc.gpsimd.dma_start(out=wb_s[:, :], in_=wb[:, :]).then_inc(isem, 16)

    w1s_s = xws_s[:, PCOLS:PCOLS + 64]
    w3d_s = wb_s[0:128, 0:128]
    w2a_s = wb_s[0:65, 128:256]
    b3b_s = wb_s[0:128, 256:257]

    # scalar: preload the ACT table (1.3us) while the input DMAs fly; the
    # scratch tile is uninitialized, the result is never read.
    scr = nc.alloc_sbuf_tensor("scr", [1, 8], F32)
    scr2 = nc.alloc_sbuf_tensor("scr2", [1, 8], F32)
    nc.scalar.activation(scr2[:, :], scr[:, :],
                         mybir.ActivationFunctionType.Relu)

    # PE warm-up: the PE runs at 1.2 GHz until its free-running activity
    # window sees ~3.4us of continuous matmul traffic, then doubles to
    # 2.4 GHz. Burn the input-DMA wait on dummy matmuls over a zeroed
    # scratch tile so the real fp32 MLP starts warm. Never read back.
    wsrc = nc.alloc_sbuf_tensor("wsrc", [128, 512], BF16)
    wps = nc.alloc_psum_tensor("wps", [128, 512], F32)
    nc.vector.memset(wsrc[:, :], 0.0)
    for _ in range(9):
        nc.tensor.matmul(wps[:, :], wsrc[:, 0:128], wsrc[:, :])

    # xws resident before the first L1 matmul (raw wait: the tile
    # scheduler's deadlock simulator doesn't model raw DMA increments, so
    # this must precede the TileContext). wb needs no explicit wait: the
    # gpsimd DMA queue is FIFO, so the tracked in-tc ones-row DMA (queued
    # after wb) lands after it, and the L2 matmul -- the first wb consumer
    # -- waits on that ones-row via h1a's dependency tracking.
    nc.tensor.wait_ge(isem, 16)

    with TileContext(nc) as tc:
        with (
            tc.tile_pool(name="sb", bufs=1) as sb,
            tc.tile_pool(name="ps", bufs=1, space="PSUM") as ps,
        ):
            pA0 = ps.tile([128, 4 * BLK], F32, name="pA0")   # pairs 0-3
            pA1 = ps.tile([128, 4 * BLK], F32, name="pA1")   # pairs 4-7
            h3 = sb.tile([128, PCOLS], F32)
            h1a = sb.tile([65, PCOLS], F32)
            h2a = sb.tile([128, PCOLS], F32)
            nc.gpsimd.dma_start(out=h1a[64:65, :], in_=ones[:, :])

            # block-diagonal stationary tiles for the attention pairs:
            # diag_p = [[h_2p, 0], [0, h_2p+1]]. Zero the off-diagonal
            # quadrants now (vector is idle during the input wait).
            diag = []
            for p in range(NP):
                dgt = sb.tile([128, 128], F32, name=f"diag{p}")
                diag.append(dgt)
                nc.vector.memset(dgt[:, :], 0.0)

            # MLP in 2 chunks of 256 packed cols (= 4 pairs each)
            MC = PCOLS // 2
            p1 = ps.tile([64, PCOLS], F32, name="p1")
            p2 = {}
            p3 = {}
            for c in range(2):
                sl = slice(c * MC, (c + 1) * MC)
                nc.tensor.matmul(p1[:, sl], w1s_s, xws_s[:, sl])
            # dep-free filler keeps the PE activity window saturated while
            # L2 waits on relu1 (the clock throttles back after idle)
            nc.tensor.matmul(wps[:, :], wsrc[:, 0:128], wsrc[:, :])
            for c in range(2):
                sl = slice(c * MC, (c + 1) * MC)
                nc.vector.tensor_scalar_max(h1a[0:64, sl], p1[:, sl], 0.0)
                p2[c] = ps.tile([128, MC], F32, name=f"p2_{c}")
                nc.tensor.matmul(p2[c], w2a_s, h1a[:, sl])
            for c in range(2):
                sl = slice(c * MC, (c + 1) * MC)
                nc.scalar.activation(h2a[:, sl], p2[c],
                                     mybir.ActivationFunctionType.Relu)
                p3[c] = ps.tile([128, MC], F32, name=f"p3_{c}")
                nc.tensor.matmul(p3[c], w3d_s, h2a[:, sl])

            nc.tensor.matmul(wps[:, :], wsrc[:, 0:128], wsrc[:, :])
            # h3 = p3 + b3 (per-partition bias); chunk 0 on vector (free
            # right after the relu1s) so the first attention pair's diagonal
            # tiles fill while the PE finishes L3 chunk 1.
            nc.vector.tensor_scalar_add(h3[:, 0:MC], p3[0], b3b_s)

            def copies(p):
                cs = slice(p * BLK, (p + 1) * BLK)
                if p == 0:
                    # both pair-0 quadrants on vector: it owns h3 chunk 0
                    # (act3c0) so attention entry never waits on the scalar
                    # queue's position for act3c1
                    nc.vector.tensor_copy(diag[p][0:64, 0:64], h3[0:64, cs])
                else:
                    nc.scalar.activation(
                        diag[p][0:64, 0:64], h3[0:64, cs],
                        mybir.ActivationFunctionType.Identity)
                nc.vector.tensor_copy(diag[p][64:128, 64:128],
                                      h3[64:128, cs])

            def attn(p):
                pa = pA0 if p < 4 else pA1
                nc.tensor.matmul(pa[:, (p % 4) * BLK:(p % 4 + 1) * BLK],
                                 diag[p], h3[:, p * BLK:(p + 1) * BLK])

            for p in range(4):
                copies(p)
            nc.scalar.activation(h3[:, MC:PCOLS], p3[1],
                                 mybir.ActivationFunctionType.Identity,
                                 bias=b3b_s, scale=1.0)
            for p in range(4):
                attn(p)
            for p in range(4, NP):
                copies(p)
                attn(p)

            # softmax pieces in pairs (3+1 per half): the final piece is a
            # single pair so the post-attention chain is short.
            bounds = [0, 4, 8]
            for q in range(2):
                q0, q1 = bounds[q], bounds[q + 1]
                npair = q1 - q0
                qs = slice(q0 * BLK, q1 * BLK)
                pa = (pA0 if q0 < 4 else pA1)[:, (q0 % 4) * BLK:
                                              ((q1 - 1) % 4 + 1) * BLK]
                r_q = sb.tile([128, npair], F32, name=f"r{q}")
                nc.vector.reduce_max(
                    r_q, pa.rearrange("p (b j) -> p b j", j=BLK),
                    axis=mybir.AxisListType.X)
                # G = exp(A - rowmax): per-partition, per-pair shift via a
                # 0-stride broadcast along j
                rrep = bass.AP(tensor=r_q.tensor, offset=r_q.offset,
                               ap=[list(r_q.ap[0]), list(r_q.ap[1]),
                                   [0, BLK]])
                d_q = sb.tile([128, npair * BLK], F32, name=f"d{q}")
                nc.vector.tensor_sub(
                    d_q.rearrange("p (b j) -> p b j", j=BLK),
                    pa.rearrange("p (b j) -> p b j", j=BLK),
                    rrep)
                band_q = sb.tile([128, npair * BLK], F32, name=f"bq{q}")
                nc.scalar.activation(band_q, d_q,
                                     mybir.ActivationFunctionType.Exp)
                nc.gpsimd.dma_start(out=bands[:, qs], in_=band_q)

    nc.compile()
    return nc


def _get_nc():
    global _NC_CACHE
    if _NC_CACHE is None:
        _NC_CACHE = build_nc()
    return _NC_CACHE


def pack_inputs(xt_core, W1, b1, W2, b2, W3, b3):
    import ml_dtypes
    bf = ml_dtypes.bfloat16
    xT = xt_core.T.astype(np.float32)          # [2, 1024]
    x_hi = xT.astype(bf)
    x_lo = (xT - x_hi.astype(np.float32)).astype(bf)
    W1_hi = W1.astype(bf)
    W1_lo = (W1 - W1_hi.astype(np.float32)).astype(bf)
    b1_hi = b1.astype(bf)
    b1_lo = (b1 - b1_hi.astype(np.float32)).astype(bf)

    def track(cols):
        """K=10 exact split stack for the given agent columns."""
        t = np.zeros((10, len(cols)), dtype=bf)
        t[0:2] = x_hi[:, cols]
        t[2:4] = x_lo[:, cols]
        t[4:6] = x_hi[:, cols]
        t[6:8] = x_lo[:, cols]
        t[8:10] = np.ones((2, len(cols)), dtype=bf)
        return t

    idx = np.arange(AGENTS).reshape(16, BLK)
    even = idx[0::2].ravel()                   # track A agent columns
    odd = idx[1::2].ravel()                    # track B
    xws = np.zeros((20, XWS_COLS), dtype=bf)
    xws[0:10, 0:PCOLS] = track(even)
    xws[10:20, 0:PCOLS] = track(odd)
    w1k = np.zeros((10, 32), dtype=bf)
    w1k[0:2] = W1_hi
    w1k[2:4] = W1_hi
    w1k[4:6] = W1_lo
    w1k[6:8] = W1_lo
    w1k[8] = b1_hi
    w1k[9] = b1_lo
    xws[0:10, PCOLS:PCOLS + 32] = w1k          # track A -> out rows 0:32
    xws[10:20, PCOLS + 32:PCOLS + 64] = w1k    # track B -> out rows 32:64

    wb = np.zeros((128, WB_COLS), dtype=np.float32)
    wb[0:64, 0:64] = W3                        # W3diag
    wb[64:128, 64:128] = W3
    wb[0:32, 128:192] = W2                     # W2a2: track A h1 rows
    wb[32:64, 192:256] = W2                    # track B
    wb[64, 128:192] = b2
    wb[64, 192:256] = b2
    wb[0:64, 256] = b3                         # b3b
    wb[64:128, 256] = b3
    return xws, wb


def kernel(x, W1, b1, W2, b2, W3, b3, sub_batches, **run_kwargs):
    global LAST_RESULT
    x = np.asarray(x)
    xt = np.ascontiguousarray(x[:, -1, :], dtype=np.float32)  # [8192, 2]
    W1 = np.asarray(W1, dtype=np.float32)
    W2 = np.asarray(W2, dtype=np.float32)
    W3 = np.asarray(W3, dtype=np.float32)
    b1 = np.asarray(b1, dtype=np.float32)
    b2 = np.asarray(b2, dtype=np.float32)
    b3 = np.asarray(b3, dtype=np.float32)

    ones = np.ones((1, PCOLS), dtype=np.float32)
    in_maps = []
    for d in range(NCORES):
        xws, wb = pack_inputs(
            xt[d * AGENTS:(d + 1) * AGENTS, :], W1, b1, W2, b2, W3, b3)
        in_maps.append({"xws": xws, "wb": wb, "ones": ones})

    nc = _get_nc()
    res = run_bass_kernel_spmd(nc, in_maps, core_ids=list(range(NCORES)),
                               **run_kwargs)
    LAST_RESULT = res

    # Device ships G = exp(A - rowmax) in dual-track layout: pair p has
    # block 2p on partitions 0:64 and block 2p+1 on 64:128. The reference
    # E = exp(A - m[j]) is G^T per block; paste transposed and normalize.
    full = np.zeros((BS, BS), dtype=np.float32)
    for d in range(NCORES):
        bd = np.asarray(res.results[d]["bands"])        # [128, 512] = G
        for p in range(NP):
            for t in range(2):
                n = d * 16 + 2 * p + t                  # global 64-row block
                G = bd[t * 64:(t + 1) * 64, p * BLK:(p + 1) * BLK]
                E = np.ascontiguousarray(G.T)
                P = E / (E.sum(axis=1, keepdims=True) + EPS)
                full[n * BLK:(n + 1) * BLK, n * BLK:(n + 1) * BLK] = P

    starts = np.asarray(sub_batches)[:, 0]
    canonical = np.array_equal(starts, np.arange(128, dtype=np.int64) * BLK)
    if not canonical:
        # General placement: extract the 64x64 blocks and scatter them at the
        # rows given by sub_batches (faithful to the reference .at[].set).
        scat = np.zeros((BS, BS), dtype=np.float32)
        for n in range(128):
            blk = full[n * BLK:(n + 1) * BLK, n * BLK:(n + 1) * BLK]
            rows = int(starts[n]) + np.arange(BLK)
            scat[np.ix_(rows, rows)] = blk
        full = scat
    return full
